# revision 1
# baseline (speedup 1.0000x reference)
"""Trainium2 Bass kernel for a causal single-head attention block.

Problem: y = softmax(mask(Q K^T / sqrt(H))) V with
  x  [B=4, T=4096, C=1024] f32,  Wq/Wk/Wv [C, H=64] f32.

Sharding (8 NeuronCores): data-parallel over B across core pairs;
within a pair, the T dimension is split by interleaved 512-row tiles
(rank r owns global q-tiles {2s+r}) so the causal work is balanced.
Each core computes Q/K/V for its own 2048 rows, the pair exchanges
K^T and V via an AllGather, and each core runs a flash-attention style
kc-outer loop over its own query rows.

The graph is identical on all 8 cores (SPMD); all rank-dependent
causality is delivered via input *data* (a sliding causal mask sheet).

Layout notes:
 - The host pre-transposes x per core to x^T [C, 2048] bf16 so the
   projections can contract over C on the partition dimension without
   any on-chip transpose of x.
 - Projections produce Q^T/K^T/V^T [64, T] directly (H on partitions),
   which is exactly the operand layout the S^T matmul wants.
 - S^T tiles are [128 k, 512 q]; exp has no running max (logits here
   are ~N(0,1), |s| < ~7, so exp is safe in f32) and the row-sum is
   folded into the PV matmul via a ones-column appended to V.
"""

import numpy as np
import ml_dtypes

import concourse.bass as bass
import concourse.bacc as bacc
import concourse.mybir as mybir
from concourse.tile import TileContext
from concourse.bass_utils import run_bass_kernel_spmd

BF16 = mybir.dt.bfloat16
F32 = mybir.dt.float32
bf16 = ml_dtypes.bfloat16

B, T, C, H = 4, 4096, 1024, 64
N_CORES = 8
TOWN = 2048          # rows owned per core
NSLOT = 4            # q-tiles of 512 rows per core
QT512 = 512
KC = 128             # k-chunk rows
NKC = T // KC        # 32 global k-chunks
W_SLOT = [8, 16, 24, 32]   # uniform kc-window per slot
V_FLAT = TOWN * H    # 131072 elements of V shard in the bounce
CC_K = H * TOWN      # K^T shard elements
CC_V = 128 * 1024    # V shard elements
MASK_W = 896 + 512   # causal mask sheet width


def build_bass():
    nc = bacc.Bacc(
        "TRN2",
        target_bir_lowering=False,
        debug=False,
        enable_asserts=False,
        num_devices=N_CORES,
    )

    xT = nc.declare_dram_parameter("xT", [C, TOWN], BF16, isOutput=False)
    wq = nc.declare_dram_parameter("wq", [C, H], BF16, isOutput=False)
    wk = nc.declare_dram_parameter("wk", [C, H], BF16, isOutput=False)
    wv = nc.declare_dram_parameter("wv", [C, H], BF16, isOutput=False)
    ident = nc.declare_dram_parameter("ident", [H, H], BF16, isOutput=False)
    mask = nc.declare_dram_parameter("mask", [128, MASK_W], BF16, isOutput=False)
    out = nc.declare_dram_parameter("out", [H, TOWN], F32, isOutput=True)

    cc_in_k = nc.dram_tensor("cc_in_k", [CC_K], BF16)
    cc_out_k = nc.dram_tensor("cc_out_k", [2 * CC_K], BF16)
    cc_in_v = nc.dram_tensor("cc_in_v", [CC_V], BF16)
    cc_out_v = nc.dram_tensor("cc_out_v", [2 * CC_V], BF16)

    with TileContext(nc) as tc:
        with (
            tc.tile_pool(name="persist", bufs=1) as pp,
            tc.tile_pool(name="work", bufs=3) as wp,
        ):
            # ---- persistent SBUF tensors ----
            xT_sb = pp.tile([128, 8, TOWN], BF16, tag="xT")
            wq_sb = pp.tile([128, 8, H], BF16, tag="wq")
            wk_sb = pp.tile([128, 8, H], BF16, tag="wk")
            wv_sb = pp.tile([128, 8, H], BF16, tag="wv")
            id_sb = pp.tile([H, H], BF16, tag="ident")
            mask_sb = pp.tile([128, MASK_W], BF16, tag="mask")
            qT_sb = pp.tile([H, TOWN], BF16, tag="qT")
            vT_own = pp.tile([H, TOWN], BF16, tag="vTown")
            kstage = pp.tile([H, TOWN], BF16, tag="kstage")
            vstage = pp.tile([128, 1024], BF16, tag="vstage")
            # rank-blocked columns: [rank0 2048 | rank1 2048]
            kT_g = pp.tile([H, T], BF16, tag="kTg")
            # V rank-blocked [128, kcb, 65]; col 64 of each chunk is ones
            vaug = pp.tile([128, NKC, H + 1], BF16, tag="vaug")
            ones_sb = pp.tile([1, H], F32, tag="ones")
            yT_sb = pp.tile([H, TOWN], F32, tag="yT")

            # ---- loads ----
            nc.sync.dma_start(
                out=xT_sb[:], in_=xT[:].rearrange("(cc p) t -> p cc t", p=128)
            )
            nc.sync.dma_start(
                out=wq_sb[:], in_=wq[:].rearrange("(cc p) h -> p cc h", p=128)
            )
            nc.sync.dma_start(
                out=wk_sb[:], in_=wk[:].rearrange("(cc p) h -> p cc h", p=128)
            )
            nc.sync.dma_start(
                out=wv_sb[:], in_=wv[:].rearrange("(cc p) h -> p cc h", p=128)
            )
            nc.sync.dma_start(out=id_sb[:], in_=ident[:])
            nc.sync.dma_start(out=mask_sb[:], in_=mask[:])
            nc.vector.memset(ones_sb[:], 1.0)
            nc.vector.memset(vaug[:, :, H : H + 1], 1.0)

            # ---- projections: Q^T, K^T, V^T for own rows ----
            with tc.tile_pool(name="proj_ps", bufs=2, space="PSUM") as proj_ps:
                def proj(w_sb, dst, sl):
                    ps = proj_ps.tile([H, QT512], F32, tag="proj", name="ps")
                    for cc in range(8):
                        nc.tensor.matmul(
                            ps[:],
                            w_sb[:, cc, :],
                            xT_sb[:, cc, sl],
                            start=(cc == 0),
                            stop=(cc == 7),
                        )
                    nc.vector.tensor_copy(dst, ps[:])

                # K first so its exchange can start ASAP
                for tt in range(NSLOT):
                    sl = slice(tt * QT512, (tt + 1) * QT512)
                    proj(wk_sb, kstage[:, sl], sl)
                nc.gpsimd.dma_start(
                    out=cc_in_k[:].rearrange("(p t) -> p t", p=H), in_=kstage[:]
                )
                nc.gpsimd.collective_compute(
                    "AllGather",
                    mybir.AluOpType.bypass,
                    replica_groups=[[2 * i, 2 * i + 1] for i in range(N_CORES // 2)],
                    ins=[cc_in_k[:]],
                    outs=[cc_out_k[:]],
                )

                for tt in range(NSLOT):
                    sl = slice(tt * QT512, (tt + 1) * QT512)
                    proj(wv_sb, vT_own[:, sl], sl)
                for tcn in range(16):
                    pst = proj_ps.tile([128, H], BF16, tag="vt")
                    nc.tensor.transpose(
                        pst[:], vT_own[:, tcn * 128 : (tcn + 1) * 128], id_sb[:]
                    )
                    nc.vector.tensor_copy(vstage[:, tcn * H : (tcn + 1) * H], pst[:])
                nc.gpsimd.dma_start(
                    out=cc_in_v[:].rearrange("(p c) -> p c", p=128), in_=vstage[:]
                )
                nc.gpsimd.collective_compute(
                    "AllGather",
                    mybir.AluOpType.bypass,
                    replica_groups=[[2 * i, 2 * i + 1] for i in range(N_CORES // 2)],
                    ins=[cc_in_v[:]],
                    outs=[cc_out_v[:]],
                )

                # Q projection overlaps the collectives
                for tt in range(NSLOT):
                    sl = slice(tt * QT512, (tt + 1) * QT512)
                    proj(wq_sb, qT_sb[:, sl], sl)

            # ---- readback of gathered K^T and V ----
            nc.gpsimd.dma_start(
                out=kT_g[:].rearrange("p (gp t) -> p gp t", gp=2),
                in_=cc_out_k[:].rearrange("(gp p t) -> p gp t", gp=2, p=H),
            )
            blkv = cc_out_v[:].rearrange("(gp p c) -> gp p c", gp=2, p=128)
            for gp in range(2):
                nc.gpsimd.dma_start(
                    out=vaug[:, gp * 16 : (gp + 1) * 16, 0:H],
                    in_=blkv[gp].rearrange("p (tc h) -> p tc h", h=H),
                )

            # ---- attention: kc-outer flash loop ----
            with (
                tc.tile_pool(name="swide", bufs=2, space="PSUM") as sp,
                tc.tile_pool(name="yacc", bufs=1, space="PSUM") as yp,
            ):
                y_acc = [
                    yp.tile([128, QT512], F32, tag=f"y{s}", name=f"y_acc{s}")
                    for s in range(NSLOT)
                ]

                for kc in range(NKC):
                    g = kc // 4
                    kcol = (g % 2) * 2048 + (g // 2) * QT512 + (kc % 4) * KC
                    kcb = (g % 2) * 16 + (g // 2) * 4 + kc % 4
                    smin = kc // 8
                    slots = list(range(smin, NSLOT))
                    for gi in range(0, len(slots), 2):
                        grp = slots[gi : gi + 2]
                        fd = QT512 * len(grp)
                        sw = sp.tile([128, 1024], F32, tag="swide")
                        for i, s in enumerate(grp):
                            nc.tensor.matmul(
                                sw[:, i * QT512 : (i + 1) * QT512],
                                kT_g[:, kcol : kcol + KC],
                                qT_sb[:, s * QT512 : (s + 1) * QT512],
                                start=True,
                                stop=True,
                            )
                        pt = wp.tile([128, 1024], BF16, tag="pt")
                        nc.scalar.activation(
                            pt[:, 0:fd],
                            sw[:, 0:fd],
                            mybir.ActivationFunctionType.Exp,
                            scale=float(H) ** -0.5,
                        )
                        for i, s in enumerate(grp):
                            psl = pt[:, i * QT512 : (i + 1) * QT512]
                            j = kc - 8 * s
                            if 0 <= j < 8:
                                o = (7 - j) * 128
                                nc.vector.tensor_mul(
                                    psl, psl, mask_sb[:, o : o + QT512]
                                )
                            nc.tensor.matmul(
                                y_acc[s][0 : H + 1, :],
                                vaug[:, kcb, :],
                                psl,
                                start=(kc == 0),
                                stop=(kc == W_SLOT[s] - 1),
                            )

                # ---- normalize and write out ----
                lsum = wp.tile([1, NSLOT * QT512], F32, tag="lsum")
                for s in range(NSLOT):
                    nc.vector.tensor_copy(
                        lsum[0:1, s * QT512 : (s + 1) * QT512], y_acc[s][H : H + 1, :]
                    )
                rec = wp.tile([1, NSLOT * QT512], F32, tag="rec")
                nc.vector.reciprocal(rec[:], lsum[:])
                for s in range(NSLOT):
                    sl = slice(s * QT512, (s + 1) * QT512)
                    bc = sp.tile([H, QT512], F32, tag="swide")
                    nc.tensor.matmul(
                        bc[:],
                        ones_sb[:],
                        rec[0:1, s * QT512 : (s + 1) * QT512],
                        start=True,
                        stop=True,
                    )
                    bc_sb = wp.tile([H, QT512], F32, tag="bcsb")
                    nc.vector.tensor_copy(bc_sb[:], bc[:])
                    nc.vector.tensor_mul(yT_sb[:, sl], y_acc[s][0:H, :], bc_sb[:])

            nc.sync.dma_start(out=out[:], in_=yT_sb[:])

    nc.compile()
    return nc


_NC_CACHE = None


def _get_nc():
    global _NC_CACHE
    if _NC_CACHE is None:
        _NC_CACHE = build_bass()
    return _NC_CACHE


def _make_in_maps(x, Wq, Wk, Wv):
    ident = np.eye(H, dtype=bf16)
    wq16, wk16, wv16 = (w.astype(bf16) for w in (Wq, Wk, Wv))
    p_idx = np.arange(128)[:, None]
    x_idx = np.arange(MASK_W)[None, :]
    masks = [
        (p_idx <= x_idx - off).astype(bf16) for off in (896, 384)
    ]  # rank 0 owns even tiles, rank 1 odd tiles
    in_maps = []
    for c in range(N_CORES):
        b, r = divmod(c, 2)
        rows = np.concatenate(
            [x[b, (2 * s + r) * QT512 : (2 * s + r + 1) * QT512] for s in range(NSLOT)]
        )
        xT_c = np.ascontiguousarray(rows.T).astype(bf16)
        in_maps.append(
            {
                "xT": xT_c,
                "wq": wq16,
                "wk": wk16,
                "wv": wv16,
                "ident": ident,
                "mask": masks[r],
            }
        )
    return in_maps


def _assemble(results):
    y = np.empty((B, T, H), dtype=np.float32)
    for c in range(N_CORES):
        b, r = divmod(c, 2)
        yt = np.asarray(results[c]["out"], dtype=np.float32).T  # [2048, 64]
        for s in range(NSLOT):
            g = 2 * s + r
            y[b, g * QT512 : (g + 1) * QT512] = yt[s * QT512 : (s + 1) * QT512]
    return y


def run(x, Wq, Wk, Wv, trace=False):
    nc = _get_nc()
    in_maps = _make_in_maps(
        np.asarray(x, np.float32),
        np.asarray(Wq, np.float32),
        np.asarray(Wk, np.float32),
        np.asarray(Wv, np.float32),
    )
    res = run_bass_kernel_spmd(nc, in_maps, core_ids=list(range(N_CORES)), trace=trace)
    return _assemble(res.results), res


def kernel(x, Wq, Wk, Wv):
    y, _ = run(x, Wq, Wk, Wv)
    return y



# revision 9
# speedup vs baseline: 1.1383x; 1.1383x over previous
"""Trainium2 Bass kernel for a causal single-head attention block.

Problem: y = softmax(mask(Q K^T / sqrt(H))) V with
  x  [B=4, T=4096, C=1024] f32,  Wq/Wk/Wv [C, H=64] f32.

Sharding (8 NeuronCores): data-parallel over B across core pairs;
within a pair, T is split by interleaved 512-row q-tiles (rank r owns
global tiles {2s+r}) so causal work is balanced.  Each core projects
Q/K/V for its own 2048 rows (K and Q fused into one [C,128] weight so
the PE runs full width), the pair exchanges K^T and V via ONE fused
AllGather, and each core runs a flash-style attention over its q rows.

The compiled graph is identical on all 8 cores (SPMD).  All rank
dependence is data:
 - `tri`  [128,512]: lower-triangle sheet for the diagonal chunks
   (identical content on both ranks thanks to local-index addressing).
 - `sel`  [128,2048]: 0/1 sheet choosing which AllGather block is the
   partner (DVE select), and killing the causal overhang items that the
   uniform graph forces on the even rank.

Structure per core (own = this core's 16 k-chunks, partner = the other
core's 16, local chunk (i,c) = chunk c of local tile i):
 - phase 1: items (s,i,c) for i<=s over OWN chunks; the i==s items are
   this rank's diagonal: width-narrowed to 512-128c with a tri mask.
 - phase 2: same index set over PARTNER chunks; i==s items are real
   (full) for the odd rank and dead for the even rank -> `sel` mask.
Exp runs on the scalar engine (groups packed to <=1024 cols), masks on
gpsimd, PV matmuls accumulate into per-slot PSUM banks with a
ones-column appended to V so row-sums land in y_acc row 64; the
division happens on the host.  PV emission lags S emission by one
group so the PE never waits on the activation.
"""

import numpy as np
import ml_dtypes

import concourse.bass as bass
import concourse.bacc as bacc
import concourse.mybir as mybir
from concourse.tile import TileContext
from concourse.bass_utils import run_bass_kernel_spmd

BF16 = mybir.dt.bfloat16
F32 = mybir.dt.float32
bf16 = ml_dtypes.bfloat16

B, T, C, H = 4, 4096, 1024, 64
N_CORES = 8
TOWN = 2048          # q rows owned per core
NSLOT = 4            # local 512-row q-tiles
QT = 512
KC = 128
CC_K = H * TOWN      # 131072 bf16 elems of K^T shard
CC_V = 128 * 1024    # 131072 bf16 elems of staged V shard
CC_IN = CC_K + CC_V


def build_items():
    """Uniform (rank-independent) attention item lists.

    item = (s, i, c, width, qoff, mask) with mask in {None,'tri','kill'}.
    phase 1 = own chunks, phase 2 = partner chunks.
    """
    ph1, ph2 = [], []
    for i in range(NSLOT):
        for c in range(4):
            for s in (3, 2, 1):
                if s > i:
                    ph1.append((s, i, c, QT, 0, None))
            ph1.append((i, i, c, QT - KC * c, KC * c, "tri"))
            for s in (3, 2, 1):
                if s > i:
                    ph2.append((s, i, c, QT, 0, None))
            ph2.append((i, i, c, QT, 0, "kill"))
    return ph1, ph2


def pack_groups(items):
    # no matmul dst may straddle a 512-col (2KB) PSUM bank boundary
    groups, cur, w = [], [], 0
    for it in items:
        if w + it[3] > 1024 or (w % 512) + it[3] > 512:
            groups.append(cur)
            cur, w = [], 0
        cur.append(it)
        w += it[3]
    if cur:
        groups.append(cur)
    return groups


def build_bass():
    nc = bacc.Bacc(
        "TRN2",
        target_bir_lowering=False,
        debug=False,
        enable_asserts=False,
        num_devices=N_CORES,
    )

    xT = nc.declare_dram_parameter("xT", [C, TOWN], BF16, isOutput=False)
    wkq = nc.declare_dram_parameter("wkq", [C, 128], BF16, isOutput=False)
    wv = nc.declare_dram_parameter("wv", [C, H], BF16, isOutput=False)
    ident = nc.declare_dram_parameter("ident", [H, H], BF16, isOutput=False)
    tri = nc.declare_dram_parameter("tri", [128, QT], BF16, isOutput=False)
    kill = nc.declare_dram_parameter("kill", [128, QT], BF16, isOutput=False)
    sel = nc.declare_dram_parameter("sel", [128, TOWN], mybir.dt.uint8, isOutput=False)
    out = nc.declare_dram_parameter("out", [H + 1, TOWN], F32, isOutput=True)

    cc_in = nc.dram_tensor("cc_in", [CC_IN], BF16)
    cc_out = nc.dram_tensor("cc_out", [2 * CC_IN], BF16)

    ph1, ph2 = build_items()
    n_items = [8 * (s + 1) for s in range(NSLOT)]

    with TileContext(nc) as tc:
        with (
            tc.tile_pool(name="persist", bufs=1) as pp,
            tc.tile_pool(name="work", bufs=3) as wp,
        ):
            # ---- persistent SBUF ----
            xT_sb = pp.tile([128, 8, TOWN], BF16, tag="xT")
            wkq_sb = pp.tile([128, 8, 128], BF16, tag="wkq")
            wv_sb = pp.tile([128, 8, H], BF16, tag="wv")
            id_sb = pp.tile([H, H], BF16, tag="ident")
            tri_sb = pp.tile([128, QT], BF16, tag="tri")
            kill_sb = pp.tile([128, QT], BF16, tag="kill")
            sel_sb = pp.tile([128, TOWN], mybir.dt.uint8, tag="sel")
            qT = pp.tile([H, TOWN], BF16, tag="qT")
            kown = pp.tile([H, TOWN], BF16, tag="kown")
            kpart = pp.tile([H, TOWN], BF16, tag="kpart")
            kA = pp.tile([H, TOWN], BF16, tag="kA")
            kB = pp.tile([H, TOWN], BF16, tag="kB")
            vA = pp.tile([128, 1024], BF16, tag="vA")
            vB = pp.tile([128, 1024], BF16, tag="vB")
            vT_own = pp.tile([H, TOWN], BF16, tag="vTown")
            # V chunks [own 0:16 | partner 16:32], col 64 = ones
            vaug = pp.tile([128, 32, H + 1], BF16, tag="vaug")

            # ---- loads (weights/masks first, then x tiles) ----
            nc.sync.dma_start(out=id_sb[:], in_=ident[:])
            nc.sync.dma_start(out=tri_sb[:], in_=tri[:])
            nc.sync.dma_start(
                out=wkq_sb[:], in_=wkq[:].rearrange("(cc p) h -> p cc h", p=128)
            )
            nc.sync.dma_start(
                out=wv_sb[:], in_=wv[:].rearrange("(cc p) h -> p cc h", p=128)
            )
            for t in range(NSLOT):
                sl = slice(t * QT, (t + 1) * QT)
                nc.sync.dma_start(
                    out=xT_sb[:, :, sl],
                    in_=xT[:, sl].rearrange("(cc p) t -> p cc t", p=128),
                )
            nc.sync.dma_start(out=kill_sb[:], in_=kill[:])
            nc.sync.dma_start(out=sel_sb[:], in_=sel[:])
            nc.vector.memset(vaug[:, :, H : H + 1], 1.0)

            # preload the exp activation table while DMAs run
            dummy = wp.tile([H, H], BF16, tag="dummy")
            nc.scalar.activation(
                dummy[:], id_sb[:], mybir.ActivationFunctionType.Exp
            )

            # ---- projections (+ PE warm-up during x load) ----
            with tc.tile_pool(name="proj_ps", bufs=2, space="PSUM") as pps:
                warm = pps.tile([128, QT], F32, tag="pkq", name="warm")
                for _ in range(7):
                    nc.tensor.matmul(
                        warm[0:H, :], id_sb[:], tri_sb[0:H, :], start=True, stop=True
                    )

                for i in range(NSLOT):
                    sl = slice(i * QT, (i + 1) * QT)
                    ps_kq = pps.tile([128, QT], F32, tag="pkq")
                    for cc in range(8):
                        nc.tensor.matmul(
                            ps_kq[:],
                            wkq_sb[:, cc, :],
                            xT_sb[:, cc, sl],
                            start=(cc == 0),
                            stop=(cc == 7),
                        )
                    nc.vector.tensor_copy(kown[:, sl], ps_kq[0:H, :])
                    nc.vector.tensor_copy(qT[:, sl], ps_kq[H:128, :])
                    ps_v = pps.tile([H, QT], F32, tag="pv")
                    for cc in range(8):
                        nc.tensor.matmul(
                            ps_v[:],
                            wv_sb[:, cc, :],
                            xT_sb[:, cc, sl],
                            start=(cc == 0),
                            stop=(cc == 7),
                        )
                    nc.vector.tensor_copy(vT_own[:, sl], ps_v[:])
                    for c in range(4):
                        pt_t = pps.tile([128, H], BF16, tag="vt")
                        nc.tensor.transpose(
                            pt_t[:],
                            vT_own[:, i * QT + c * KC : i * QT + (c + 1) * KC],
                            id_sb[:],
                        )
                        nc.vector.tensor_copy(vaug[:, 4 * i + c, 0:H], pt_t[:])

            # ---- fused K+V AllGather within the pair ----
            nc.gpsimd.dma_start(
                out=cc_in[0:CC_K].rearrange("(p t) -> p t", p=H), in_=kown[:]
            )
            nc.gpsimd.dma_start(
                out=cc_in[CC_K:CC_IN].rearrange("(p tc h) -> p tc h", p=128, tc=16),
                in_=vaug[:, 0:16, 0:H],
            )
            nc.gpsimd.collective_compute(
                "AllGather",
                mybir.AluOpType.bypass,
                replica_groups=[[2 * i, 2 * i + 1] for i in range(N_CORES // 2)],
                ins=[cc_in[:]],
                outs=[cc_out[:]],
            )
            # readback both blocks (sync queue), then rank-select (vector)
            nc.sync.dma_start(
                out=kA[:], in_=cc_out[0:CC_K].rearrange("(p t) -> p t", p=H)
            )
            nc.sync.dma_start(
                out=kB[:],
                in_=cc_out[CC_IN : CC_IN + CC_K].rearrange("(p t) -> p t", p=H),
            )
            nc.sync.dma_start(
                out=vA[:], in_=cc_out[CC_K:CC_IN].rearrange("(p c) -> p c", p=128)
            )
            nc.sync.dma_start(
                out=vB[:],
                in_=cc_out[CC_IN + CC_K : 2 * CC_IN].rearrange("(p c) -> p c", p=128),
            )
            # sel==1 -> partner is block A (odd rank), else block B
            nc.vector.select(kpart[:], sel_sb[0:H, :], kA[:], kB[:])
            nc.vector.select(
                vaug[:, 16:32, 0:H],
                sel_sb[:, 0:1024].rearrange("p (tc h) -> p tc h", h=H),
                vA[:].rearrange("p (tc h) -> p tc h", h=H),
                vB[:].rearrange("p (tc h) -> p tc h", h=H),
            )

            # ---- attention ----
            with (
                tc.tile_pool(name="swide", bufs=2, space="PSUM") as sp,
                tc.tile_pool(name="yacc", bufs=1, space="PSUM") as yp,
            ):
                y_acc = [
                    yp.tile([128, QT], F32, tag=f"y{s}", name=f"y_acc{s}")
                    for s in range(NSLOT)
                ]
                cnt = [0] * NSLOT

                def emit_pv(pend):
                    pt, metas = pend
                    for (s, i, c, w, qoff, mask), off, vbase in metas:
                        cnt[s] += 1
                        nc.tensor.matmul(
                            y_acc[s][0 : H + 1, qoff : qoff + w],
                            vaug[:, vbase + 4 * i + c, :],
                            pt[:, off : off + w],
                            start=(cnt[s] == 1),
                            stop=(cnt[s] == n_items[s]),
                        )
                        if cnt[s] == n_items[s]:
                            ysb = wp.tile([H + 1, QT], F32, tag="ysb")
                            nc.vector.tensor_copy(ysb[:], y_acc[s][0 : H + 1, :])
                            nc.sync.dma_start(
                                out=out[:, s * QT : (s + 1) * QT], in_=ysb[:]
                            )

                pend = None
                for phase, groups in (
                    (1, pack_groups(ph1)),
                    (2, pack_groups(ph2)),
                ):
                    kb = kown if phase == 1 else kpart
                    vbase = 0 if phase == 1 else 16
                    for g in groups:
                        sw = sp.tile([128, 1024], F32, tag="swide")
                        metas, cur = [], 0
                        for it in g:
                            s, i, c, w, qoff, mask = it
                            kcol = KC * (4 * i + c)
                            nc.tensor.matmul(
                                sw[:, cur : cur + w],
                                kb[:, kcol : kcol + KC],
                                qT[:, s * QT + qoff : s * QT + qoff + w],
                                start=True,
                                stop=True,
                            )
                            metas.append((it, cur, vbase))
                            cur += w
                        if pend is not None:
                            emit_pv(pend)
                        pt = wp.tile([128, 1024], BF16, tag="pt")
                        nc.scalar.activation(
                            pt[:, 0:cur],
                            sw[:, 0:cur],
                            mybir.ActivationFunctionType.Exp,
                            scale=float(H) ** -0.5,
                        )
                        for (s, i, c, w, qoff, mask), off, _ in metas:
                            if mask == "tri":
                                nc.gpsimd.tensor_mul(
                                    pt[:, off : off + w],
                                    pt[:, off : off + w],
                                    tri_sb[:, 0:w],
                                )
                            elif mask == "kill":
                                nc.gpsimd.tensor_mul(
                                    pt[:, off : off + w],
                                    pt[:, off : off + w],
                                    kill_sb[:, 0:w],
                                )
                        pend = (pt, metas)
                emit_pv(pend)

    nc.compile()
    return nc


_NC_CACHE = None


def _get_nc():
    global _NC_CACHE
    if _NC_CACHE is None:
        _NC_CACHE = build_bass()
    return _NC_CACHE


def _make_in_maps(x, Wq, Wk, Wv):
    ident = np.eye(H, dtype=bf16)
    wkq = np.concatenate([Wk, Wq], axis=1).astype(bf16)
    wv16 = Wv.astype(bf16)
    p = np.arange(128)[:, None]
    xx = np.arange(QT)[None, :]
    tri = (p <= xx).astype(bf16)
    kills = [np.zeros((128, QT), dtype=bf16), np.ones((128, QT), dtype=bf16)]
    sels = [
        np.zeros((128, TOWN), dtype=np.uint8),
        np.ones((128, TOWN), dtype=np.uint8),
    ]
    in_maps = []
    for c in range(N_CORES):
        b, r = divmod(c, 2)
        rows = np.concatenate(
            [x[b, (2 * s + r) * QT : (2 * s + r + 1) * QT] for s in range(NSLOT)]
        )
        xT_c = np.ascontiguousarray(rows.T).astype(bf16)
        in_maps.append(
            {
                "xT": xT_c,
                "wkq": wkq,
                "wv": wv16,
                "ident": ident,
                "tri": tri,
                "kill": kills[r],
                "sel": sels[r],
            }
        )
    return in_maps


def _assemble(results):
    y = np.empty((B, T, H), dtype=np.float32)
    for c in range(N_CORES):
        b, r = divmod(c, 2)
        o = np.asarray(results[c]["out"], dtype=np.float32)  # [65, 2048]
        yt = o[0:H] / o[H : H + 1]
        for s in range(NSLOT):
            g = 2 * s + r
            y[b, g * QT : (g + 1) * QT] = yt[:, s * QT : (s + 1) * QT].T
    return y


def run(x, Wq, Wk, Wv, trace=False):
    nc = _get_nc()
    in_maps = _make_in_maps(
        np.asarray(x, np.float32),
        np.asarray(Wq, np.float32),
        np.asarray(Wk, np.float32),
        np.asarray(Wv, np.float32),
    )
    res = run_bass_kernel_spmd(nc, in_maps, core_ids=list(range(N_CORES)), trace=trace)
    return _assemble(res.results), res


def kernel(x, Wq, Wk, Wv):
    y, _ = run(x, Wq, Wk, Wv)
    return y
